# revision 4
# baseline (speedup 1.0000x reference)
"""Capsule-routing kernel for Trainium2, 8-core batch-parallel.

Reference computation (per example, In=4096, D=256, N=16, K=16, routings=3):
    u_hat = (x @ W).reshape(In, N, K)            # [In, 256] with m = n*16+k
    b = 0
    for j in range(3):
        c = softmax(b, axis=n)                   # [In, N]
        outputs = squash(sum_i c[i,n] u_hat[i,n,:])   # [N, K]
        if j < 2: b[i,n] = sum_k outputs[n,k] u_hat[i,n,k]

Key algebraic restructure: u_hat is never materialized.
    acc = C^T (X W) = (C^T X) W        -> G = X^T C  [D,16], acc = G^T W  [16,256]
    b   = (X W) S   = X (W S)          -> WS = W^T-tiles @ S [D,16], b = X WS
so the only big PE work is 64 tile transposes of xT (bf16) per example to
get x in [i,d] layout; the routing itself is ~7k PE cycles/example.

Device strategy per core (4 examples):
  - host supplies xT [2,128,In] in bf16 (d on partitions) per example
  - PE transposes xT tiles -> x tiles [i,d] (bf16 via PSUM, copied out by
    DVE/Act/Pool round-robin)
  - G:   64 mm  (stationary x-tile,  moving c   [.,16])  per routing iter
  - acc:  2 mm  (stationary G bf16,  moving Wt  [.,256])
  - WS:   4 mm  (stationary WT-tile, moving S   [.,16])
  - b:   64 mm  (stationary xT-tile, moving WS  [.,16])
  - softmax over n: exp (ScalarE, straight from PSUM) + reduce/recip/mul (DVE)
  - squash: bmask mul + Square-accum + bit-trick rsqrt (as before)
"""

import sys
from contextlib import ExitStack

sys.path.insert(0, "/opt/trn_rl_repo")

import numpy as np
import ml_dtypes

import concourse.bass as bass
import concourse.mybir as mybir
import concourse.tile as tile
from concourse import bacc
from concourse.bass_utils import run_bass_kernel_spmd

F32 = mybir.dt.float32
BF16 = mybir.dt.bfloat16
U32 = mybir.dt.uint32

N_CORES = 8
B = 32
IN = 4096
D = 256
N = 16
K = 16
M = N * K  # 256
EPS = 1e-7


def build_kernel(n_ex=4, n_tiles=32, routings=3, copy_rot=("v", "a", "v", "a",
                                                          "v", "a", "v", "a")):
    """Build the per-core Bass module. In = n_tiles*128."""
    In = n_tiles * 128
    nc = bacc.Bacc("TRN2", target_bir_lowering=False, debug=False,
                   num_devices=N_CORES)

    # DRAM I/O
    xT_d = nc.dram_tensor("xT", [n_ex, 2, 128, In], BF16, kind="ExternalInput")
    Wt_d = nc.dram_tensor("Wt", [2, 128, M], BF16, kind="ExternalInput")
    WTt_d = nc.dram_tensor("WTt", [2, 2, 128, 128], BF16, kind="ExternalInput")
    id128_d = nc.dram_tensor("id128", [128, 128], BF16, kind="ExternalInput")
    ones16_d = nc.dram_tensor("ones16", [128, N], BF16, kind="ExternalInput")
    bmask_d = nc.dram_tensor("bmask", [N, M], F32, kind="ExternalInput")
    id16_d = nc.dram_tensor("id16", [N, N], F32, kind="ExternalInput")
    out_d = nc.dram_tensor("out", [n_ex, N, K], F32, kind="ExternalOutput")

    with tile.TileContext(nc) as tc, ExitStack() as ctx:
        # ---- pools ----
        const_pool = ctx.enter_context(tc.tile_pool(name="consts", bufs=1))
        xT_pool = ctx.enter_context(tc.tile_pool(name="xT", bufs=n_ex))
        x_pool = ctx.enter_context(tc.tile_pool(name="x", bufs=n_ex))
        c_pool = ctx.enter_context(tc.tile_pool(name="c", bufs=n_ex))
        sm_pool = ctx.enter_context(tc.tile_pool(name="sm", bufs=2))
        small_pool = ctx.enter_context(tc.tile_pool(name="small", bufs=4))
        out_pool = ctx.enter_context(tc.tile_pool(name="outstage", bufs=1))

        ps_stage = ctx.enter_context(
            tc.tile_pool(name="ps_stage", bufs=2, space="PSUM"))
        ps_b = ctx.enter_context(tc.tile_pool(name="ps_b", bufs=2, space="PSUM"))
        ps_sm = ctx.enter_context(tc.tile_pool(name="ps_sm", bufs=2, space="PSUM"))
        ps_acc = ctx.enter_context(tc.tile_pool(name="ps_acc", bufs=2, space="PSUM"))

        # ---- constants ----
        Wt = const_pool.tile([128, 2, M], BF16, tag="Wt")
        nc.sync.dma_start(Wt[:], Wt_d.ap().rearrange("c p m -> p c m"))
        WTt = const_pool.tile([128, 2, 2, 128], BF16, tag="WTt")
        nc.sync.dma_start(WTt[:], WTt_d.ap().rearrange("a b p f -> p a b f"))
        id128 = const_pool.tile([128, 128], BF16, tag="id128")
        nc.sync.dma_start(id128[:], id128_d[:])
        ones16 = const_pool.tile([128, N], BF16, tag="ones16")
        nc.sync.dma_start(ones16[:], ones16_d[:])
        bmask = const_pool.tile([N, M], F32, tag="bmask")
        nc.sync.dma_start(bmask[:], bmask_d[:])
        id16 = const_pool.tile([N, N], F32, tag="id16")
        nc.sync.dma_start(id16[:], id16_d[:])

        out_stage = out_pool.tile([N, n_ex * K], F32, tag="outst")

        A = mybir.AluOpType
        ACT = mybir.ActivationFunctionType

        xT_tiles = [None] * n_ex
        x_tiles = [None] * n_ex

        def emit_dma(e):
            xT = xT_pool.tile([128, 2, In], BF16, tag="xT")
            nc.sync.dma_start(xT[:, 0, :], xT_d[e, 0])
            nc.sync.dma_start(xT[:, 1, :], xT_d[e, 1])
            xT_tiles[e] = xT

        def emit_transpose(e):
            """xT [d,i] tiles -> x [i,d] tiles via PE, staged through PSUM."""
            xT = xT_tiles[e]
            x_sb = x_pool.tile([128, n_tiles, 2, 128], BF16, tag="x")
            # 64 transposes in batches of 8 per PSUM bank
            pairs = [(t, dc) for t in range(n_tiles) for dc in range(2)]
            for bi in range(len(pairs) // 8):
                stage = ps_stage.tile([128, 8, 128], BF16, tag="stage")
                for s in range(8):
                    t, dc = pairs[8 * bi + s]
                    nc.tensor.matmul(stage[:, s, :],
                                     xT[:, dc, 128 * t:128 * (t + 1)],
                                     id128[:], is_transpose=True,
                                     start=True, stop=True,
                                     skip_group_check=True)
                dst = x_sb[:, 4 * bi:4 * (bi + 1), :, :]
                eng = copy_rot[bi % len(copy_rot)]
                if eng == "v":
                    nc.vector.tensor_copy(
                        dst.rearrange("p a b f -> p (a b f)"), stage[:])
                elif eng == "a":
                    nc.scalar.copy(
                        dst.rearrange("p a b f -> p (a b f)"), stage[:])
                else:
                    nc.gpsimd.tensor_copy(
                        dst.rearrange("p a b f -> p (a b f)"), stage[:])
            x_tiles[e] = x_sb

        c_cur = [None] * n_ex

        def emit_routing_step(e, j):
            xT = xT_tiles[e]
            x_sb = x_tiles[e]

            # --- G = X^T C : [128(d-chunk), 2, 16] ---
            gsw = ps_sm.tile([128, 96], F32, tag="gsw")
            for dc in range(2):
                for t in range(n_tiles):
                    c_ap = ones16[:] if j == 0 else c_cur[e][:, t, :]
                    nc.tensor.matmul(gsw[:, 16 * dc:16 * (dc + 1)],
                                     x_sb[:, t, dc, :], c_ap,
                                     start=(t == 0), stop=(t == n_tiles - 1),
                                     skip_group_check=True)
            G_sb = small_pool.tile([128, 2, N], BF16, tag="G")
            nc.vector.tensor_copy(G_sb[:].rearrange("p a b -> p (a b)"),
                                  gsw[:, 0:32])

            # --- acc = G^T W : [16, 256] ---
            acc_ps = ps_acc.tile([N, M], F32, tag="acc")
            for dc in range(2):
                nc.tensor.matmul(acc_ps[:], G_sb[:, dc, :], Wt[:, dc, :],
                                 start=(dc == 0), stop=(dc == 1))

            # --- squash ---
            o_full = small_pool.tile([N, M], F32, tag="o_full")
            nc.scalar.copy(o_full[:], acc_ps[:])
            om = small_pool.tile([N, M], F32, tag="om")
            nrm2 = small_pool.tile([N, 1], F32, tag="nrm2")
            sq = small_pool.tile([N, M], F32, tag="sq")
            nc.vector.tensor_mul(om[:], o_full[:], bmask[:])
            nc.scalar.activation(sq[:], om[:], ACT.Square, accum_out=nrm2[:])
            # rinv = 1/sqrt(nrm2 + eps) via bit-trick + Newton steps
            xe = small_pool.tile([N, 1], F32, tag="xe")
            nc.vector.tensor_scalar_add(xe[:], nrm2[:], EPS)
            sbits = small_pool.tile([N, 1], U32, tag="sbits")
            nc.vector.tensor_scalar(sbits[:], xe[:].bitcast(U32), 1, None,
                                    op0=A.logical_shift_right)
            ybits = small_pool.tile([N, 1], U32, tag="ybits")
            nc.vector.tensor_scalar(ybits[:], sbits[:], -1.0,
                                    float(0x5F3759DF), op0=A.mult, op1=A.add)
            y = ybits[:].bitcast(F32)
            t1 = small_pool.tile([N, 1], F32, tag="t1")
            t2 = small_pool.tile([N, 1], F32, tag="t2")
            rinv = small_pool.tile([N, 1], F32, tag="rinv")
            n_newton = 2 if j == routings - 1 else 1
            for it in range(n_newton):
                nc.vector.tensor_mul(t1[:], xe[:], y)
                nc.vector.tensor_mul(t2[:], t1[:], y)
                nc.vector.tensor_scalar(t2[:], t2[:], -0.5, 1.5,
                                        op0=A.mult, op1=A.add)
                dst = rinv if it == n_newton - 1 else small_pool.tile(
                    [N, 1], F32, tag="ynext")
                nc.vector.tensor_mul(dst[:], t2[:], y)
                y = dst[:]
            o_n = small_pool.tile([N, M], F32, tag="o_n")
            nc.scalar.mul(o_n[:], om[:], rinv[:])

            if j == routings - 1:
                # final extraction: out[n,k] = sum_g o_n[n, g*16+k]
                nc.vector.tensor_reduce(
                    out_stage[:, K * e:K * (e + 1)],
                    o_n[:].rearrange("p (g k) -> p k g", k=K),
                    axis=mybir.AxisListType.X, op=A.add)
                return

            # --- S = o_n^T (block diagonal by construction) : [128, 2, 16] ---
            for mc in range(2):
                nc.tensor.transpose(gsw[:, 32 + 16 * mc:32 + 16 * (mc + 1)],
                                    o_n[:, 128 * mc:128 * (mc + 1)], id16[:])
            S_sb = small_pool.tile([128, 2, N], BF16, tag="S")
            nc.vector.tensor_copy(S_sb[:].rearrange("p a b -> p (a b)"),
                                  gsw[:, 32:64])

            # --- WS = W^T-tiles @ S : [128(d-chunk), 2, 16] ---
            for dc in range(2):
                for mc in range(2):
                    nc.tensor.matmul(gsw[:, 64 + 16 * dc:64 + 16 * (dc + 1)],
                                     WTt[:, mc, dc, :], S_sb[:, mc, :],
                                     start=(mc == 0), stop=(mc == 1),
                                     skip_group_check=True)
            WS_sb = small_pool.tile([128, 2, N], BF16, tag="WS")
            nc.vector.tensor_copy(WS_sb[:].rearrange("p a b -> p (a b)"),
                                  gsw[:, 64:96])

            # --- b = X WS : [128, n_tiles, 16] ---
            b_ps = ps_b.tile([128, n_tiles, N], F32, tag="b_ps")
            for t in range(n_tiles):
                for dc in range(2):
                    nc.tensor.matmul(b_ps[:, t, :],
                                     xT[:, dc, 128 * t:128 * (t + 1)],
                                     WS_sb[:, dc, :],
                                     start=(dc == 0), stop=(dc == 1),
                                     skip_group_check=True)

            # --- softmax over n (exp straight from PSUM) ---
            e_all = sm_pool.tile([128, n_tiles, N], F32, tag="e_all")
            nc.scalar.activation(e_all[:], b_ps[:], ACT.Exp)
            s_sum = sm_pool.tile([128, n_tiles], F32, tag="s_sum")
            nc.vector.tensor_reduce(s_sum[:], e_all[:],
                                    axis=mybir.AxisListType.X, op=A.add)
            s_r = sm_pool.tile([128, n_tiles], F32, tag="s_r")
            nc.vector.reciprocal(s_r[:], s_sum[:])
            c_new = c_pool.tile([128, n_tiles, N], BF16, tag="c_all")
            nc.vector.tensor_mul(c_new[:], e_all[:],
                                 s_r[:].to_broadcast([128, n_tiles, N]))
            c_cur[e] = c_new

        # ======== emission schedule (software pipelining) ========
        for e in range(n_ex):
            emit_dma(e)
        emit_transpose(0)
        emit_transpose(1)
        for j in range(routings):
            emit_routing_step(0, j)
            emit_routing_step(1, j)
            if j < 2:
                emit_transpose(2 + j)
        for j in range(routings):
            emit_routing_step(2, j)
            emit_routing_step(3, j)

        # ======== store outputs ========
        nc.sync.dma_start(out_d.ap().rearrange("e n k -> n e k"),
                          out_stage[:].rearrange("p (e k) -> p e k", k=K))

    nc.compile()
    return nc


_NC_CACHE = {}


def _get_nc(**kw):
    key = tuple(sorted(kw.items()))
    if key not in _NC_CACHE:
        _NC_CACHE[key] = build_kernel(**kw)
    return _NC_CACHE[key]


def make_const_inputs():
    ones16 = np.full((128, N), 1.0 / N, dtype=ml_dtypes.bfloat16)
    bmask = np.zeros((N, M), dtype=np.float32)
    for n in range(N):
        bmask[n, n * K:(n + 1) * K] = 1.0
    id16 = np.eye(N, dtype=np.float32)
    id128 = np.eye(128, dtype=ml_dtypes.bfloat16)
    return ones16, bmask, id16, id128


def kernel(x, W, num_capsule=None, dim_capsule=None, routings=None, **_):
    x = np.asarray(x, dtype=np.float32)
    W = np.asarray(W, dtype=np.float32)
    assert x.shape == (B, IN, D), x.shape

    nc = _get_nc()
    ones16, bmask, id16, id128 = make_const_inputs()
    Wtb = np.ascontiguousarray(W[0].reshape(2, 128, M)).astype(ml_dtypes.bfloat16)
    WT = np.ascontiguousarray(W[0].T)  # [m, d]
    WTtb = np.ascontiguousarray(
        WT.reshape(2, 128, 2, 128).transpose(0, 2, 1, 3)).astype(
            ml_dtypes.bfloat16)

    n_per = B // N_CORES
    in_maps = []
    for c in range(N_CORES):
        xs = x[c * n_per:(c + 1) * n_per]              # [4, 4096, 256]
        xT = np.ascontiguousarray(
            xs.transpose(0, 2, 1)).reshape(n_per, 2, 128, IN).astype(
                ml_dtypes.bfloat16)
        in_maps.append({"xT": xT, "Wt": Wtb, "WTt": WTtb, "id128": id128,
                        "ones16": ones16, "bmask": bmask, "id16": id16})

    res = run_bass_kernel_spmd(nc, in_maps, core_ids=list(range(N_CORES)))
    out = np.concatenate([r["out"] for r in res.results], axis=0)
    return out.astype(np.float32)


# revision 8
# speedup vs baseline: 1.1223x; 1.1223x over previous
"""Capsule-routing kernel for Trainium2, 8-core batch-parallel.

Reference computation (per example, In=4096, D=256, N=16, K=16, routings=3):
    u_hat = (x @ W).reshape(In, N, K)            # [In, 256] with m = n*16+k
    b = 0
    for j in range(3):
        c = softmax(b, axis=n)                   # [In, N]
        outputs = squash(sum_i c[i,n] u_hat[i,n,:])   # [N, K]
        if j < 2: b[i,n] = sum_k outputs[n,k] u_hat[i,n,k]

Key algebraic restructure: u_hat is never materialized.
    acc = C^T (X W) = (C^T X) W        -> G = X^T C  [D,16], acc = G^T W  [16,256]
    b   = (X W) S   = X (W S)          -> WS = WT-tiles @ S [D,16], b = X WS
so the only big PE work is 64 tile transposes of xT (bf16) per example to
get x in [i,d] layout; the routing itself is ~7k PE cycles/example.

Schedule: all 4 examples' routing is emitted phase-interleaved so that
cross-engine dependency latency amortizes 4-wide and every engine queue
always has ready work behind a stalled head (engines execute in order).
"""

import sys
from contextlib import ExitStack

sys.path.insert(0, "/opt/trn_rl_repo")

import numpy as np
import ml_dtypes

import concourse.bass as bass
import concourse.mybir as mybir
import concourse.tile as tile
from concourse import bacc
from concourse.bass_utils import run_bass_kernel_spmd

F32 = mybir.dt.float32
BF16 = mybir.dt.bfloat16
U32 = mybir.dt.uint32

N_CORES = 8
B = 32
IN = 4096
D = 256
N = 16
K = 16
M = N * K  # 256
EPS = 1e-7


def build_kernel(n_ex=4, n_tiles=32, routings=3,
                 copy_rot=("v", "a", "v", "a", "v", "a", "v", "a")):
    """Build the per-core Bass module. In = n_tiles*128."""
    In = n_tiles * 128
    nc = bacc.Bacc("TRN2", target_bir_lowering=False, debug=False,
                   num_devices=N_CORES)

    # DRAM I/O
    xT_d = nc.dram_tensor("xT", [n_ex, 2, 128, In], BF16, kind="ExternalInput")
    Wt_d = nc.dram_tensor("Wt", [2, 128, M], BF16, kind="ExternalInput")
    WTt_d = nc.dram_tensor("WTt", [2, 2, 128, 128], BF16, kind="ExternalInput")
    id128_d = nc.dram_tensor("id128", [128, 128], BF16, kind="ExternalInput")
    ones16_d = nc.dram_tensor("ones16", [128, N], BF16, kind="ExternalInput")
    bmask_d = nc.dram_tensor("bmask", [N, M], F32, kind="ExternalInput")
    id16_d = nc.dram_tensor("id16", [N, N], F32, kind="ExternalInput")
    out_d = nc.dram_tensor("out", [n_ex, N, K], F32, kind="ExternalOutput")

    with tile.TileContext(nc) as tc, ExitStack() as ctx:
        # ---- pools ----
        const_pool = ctx.enter_context(tc.tile_pool(name="consts", bufs=1))
        xT_pool = ctx.enter_context(tc.tile_pool(name="xT", bufs=n_ex))
        x_pool = ctx.enter_context(tc.tile_pool(name="x", bufs=n_ex))
        c_pool = ctx.enter_context(tc.tile_pool(name="c", bufs=n_ex))
        sm_pool = ctx.enter_context(tc.tile_pool(name="sm", bufs=n_ex))
        small_pool = ctx.enter_context(tc.tile_pool(name="small", bufs=n_ex))
        out_pool = ctx.enter_context(tc.tile_pool(name="outstage", bufs=1))
        # single PSUM pool; per-tag bufs keep the total at 13.5KB <= 8 banks
        ps = ctx.enter_context(tc.tile_pool(name="ps", bufs=1, space="PSUM"))

        # ---- constants ----
        Wt = const_pool.tile([128, 2, M], BF16, tag="Wt")
        nc.sync.dma_start(Wt[:], Wt_d.ap().rearrange("c p m -> p c m"))
        WTt = const_pool.tile([128, 2, 2, 128], BF16, tag="WTt")
        nc.sync.dma_start(WTt[:], WTt_d.ap().rearrange("a b p f -> p a b f"))
        id128 = const_pool.tile([128, 128], BF16, tag="id128")
        nc.sync.dma_start(id128[:], id128_d[:])
        ones16 = const_pool.tile([128, N], BF16, tag="ones16")
        nc.sync.dma_start(ones16[:], ones16_d[:])
        bmask = const_pool.tile([N, M], F32, tag="bmask")
        nc.sync.dma_start(bmask[:], bmask_d[:])
        id16 = const_pool.tile([N, N], F32, tag="id16")
        nc.sync.dma_start(id16[:], id16_d[:])

        out_stage = out_pool.tile([N, n_ex * K], F32, tag="outst")

        A = mybir.AluOpType
        ACT = mybir.ActivationFunctionType

        xT_t = [None] * n_ex
        x_t = [None] * n_ex
        c_t = [None] * n_ex
        st = {}  # per-example per-stage small tiles

        def emit_dma(e):
            xT = xT_pool.tile([128, 2, In], BF16, tag="xT")
            nc.sync.dma_start(xT[:, 0, :], xT_d[e, 0])
            nc.sync.dma_start(xT[:, 1, :], xT_d[e, 1])
            xT_t[e] = xT

        def emit_transpose(e):
            """xT [d,i] tiles -> x [i,d] tiles via PE, staged through PSUM."""
            xT = xT_t[e]
            x_sb = x_pool.tile([128, n_tiles, 2, 128], BF16, tag="x")
            pairs = [(t, dc) for t in range(n_tiles) for dc in range(2)]
            for bi in range(len(pairs) // 8):
                stage = ps.tile([128, 8, 128], BF16, tag="stage", bufs=2)
                for s in range(8):
                    t, dc = pairs[8 * bi + s]
                    nc.tensor.matmul(stage[:, s, :],
                                     xT[:, dc, 128 * t:128 * (t + 1)],
                                     id128[:], is_transpose=True,
                                     start=True, stop=True,
                                     skip_group_check=True)
                dst = x_sb[:, 4 * bi:4 * (bi + 1), :, :].rearrange(
                    "p a b f -> p (a b f)")
                if copy_rot[bi % len(copy_rot)] == "v":
                    nc.vector.tensor_copy(dst, stage[:])
                else:
                    nc.scalar.copy(dst, stage[:])
            x_t[e] = x_sb

        # ---------- routing phases (each emitted for all examples) ----------
        def ph_G(e, j):
            # one PSUM bank per routing step: g [0:32], S [32:64], WS [64:96],
            # acc (partitions 0:16) [96:352]
            misc = ps.tile([128, 512], F32, tag="misc", bufs=4)
            st[e, "misc"] = misc
            for dc in range(2):
                for t in range(n_tiles):
                    c_ap = ones16[:] if j == 0 else c_t[e][:, t, :]
                    nc.tensor.matmul(misc[:, 16 * dc:16 * (dc + 1)],
                                     x_t[e][:, t, dc, :], c_ap,
                                     start=(t == 0), stop=(t == n_tiles - 1),
                                     skip_group_check=True)

        def ph_Gcopy(e, j):
            G_sb = small_pool.tile([128, 2, N], BF16, tag="G")
            nc.vector.tensor_copy(G_sb[:].rearrange("p a b -> p (a b)"),
                                  st[e, "misc"][:, 0:32])
            st[e, "G"] = G_sb

        def ph_acc(e, j):
            acc_ps = st[e, "misc"][0:N, 96:96 + M]
            for dc in range(2):
                nc.tensor.matmul(acc_ps, st[e, "G"][:, dc, :], Wt[:, dc, :],
                                 start=(dc == 0), stop=(dc == 1),
                                 skip_group_check=True)
            st[e, "acc"] = acc_ps

        def ph_om(e, j):
            om = small_pool.tile([N, M], F32, tag="om")
            nc.vector.tensor_mul(om[:], st[e, "acc"], bmask[:])
            st[e, "om"] = om

        def ph_sq(e, j):
            nrm2 = small_pool.tile([N, 1], F32, tag="nrm2")
            sq = small_pool.tile([N, M], F32, tag="sq")
            nc.scalar.activation(sq[:], st[e, "om"][:], ACT.Square,
                                 accum_out=nrm2[:])
            st[e, "nrm2"] = nrm2

        def ph_rsqrt(e, j):
            # rinv = 1/sqrt(nrm2 + eps) via bit-trick + Newton steps (DVE only;
            # Act Sqrt would thrash the activation table against Exp)
            xe = small_pool.tile([N, 1], F32, tag="xe")
            nc.vector.tensor_scalar_add(xe[:], st[e, "nrm2"][:], EPS)
            sbits = small_pool.tile([N, 1], U32, tag="sbits")
            nc.vector.tensor_scalar(sbits[:], xe[:].bitcast(U32), 1, None,
                                    op0=A.logical_shift_right)
            ybits = small_pool.tile([N, 1], U32, tag="ybits")
            nc.vector.tensor_scalar(ybits[:], sbits[:], -1.0,
                                    float(0x5F3759DF), op0=A.mult, op1=A.add)
            y = ybits[:].bitcast(F32)
            t1 = small_pool.tile([N, 1], F32, tag="t1")
            t2 = small_pool.tile([N, 1], F32, tag="t2")
            rinv = small_pool.tile([N, 1], F32, tag="rinv")
            n_newton = 2 if j == routings - 1 else 1
            for it in range(n_newton):
                nc.vector.tensor_mul(t1[:], xe[:], y)
                nc.vector.tensor_mul(t2[:], t1[:], y)
                nc.vector.tensor_scalar(t2[:], t2[:], -0.5, 1.5,
                                        op0=A.mult, op1=A.add)
                dst = rinv if it == n_newton - 1 else small_pool.tile(
                    [N, 1], F32, tag="ynext")
                nc.vector.tensor_mul(dst[:], t2[:], y)
                y = dst[:]
            st[e, "rinv"] = rinv

        def ph_on(e, j):
            o_n = small_pool.tile([N, M], F32, tag="o_n")
            nc.scalar.mul(o_n[:], st[e, "om"][:], st[e, "rinv"][:])
            st[e, "o_n"] = o_n

        def ph_S(e, j):
            misc = st[e, "misc"]
            for mc in range(2):
                nc.tensor.transpose(misc[:, 32 + 16 * mc:32 + 16 * (mc + 1)],
                                    st[e, "o_n"][:, 128 * mc:128 * (mc + 1)],
                                    id16[:])

        def ph_Scopy(e, j):
            S_sb = small_pool.tile([128, 2, N], BF16, tag="S")
            nc.vector.tensor_copy(S_sb[:].rearrange("p a b -> p (a b)"),
                                  st[e, "misc"][:, 32:64])
            st[e, "S"] = S_sb

        def ph_WS(e, j):
            misc = st[e, "misc"]
            for dc in range(2):
                for mc in range(2):
                    nc.tensor.matmul(misc[:, 64 + 16 * dc:64 + 16 * (dc + 1)],
                                     WTt[:, mc, dc, :],
                                     st[e, "S"][:, mc, :],
                                     start=(mc == 0), stop=(mc == 1),
                                     skip_group_check=True)

        def ph_WScopy(e, j):
            WS_sb = small_pool.tile([128, 2, N], BF16, tag="WS")
            nc.vector.tensor_copy(WS_sb[:].rearrange("p a b -> p (a b)"),
                                  st[e, "misc"][:, 64:96])
            st[e, "WS"] = WS_sb

        def ph_b(e, j):
            b_ps = ps.tile([128, n_tiles, N], F32, tag="b", bufs=2)
            for t in range(n_tiles):
                for dc in range(2):
                    nc.tensor.matmul(b_ps[:, t, :],
                                     xT_t[e][:, dc, 128 * t:128 * (t + 1)],
                                     st[e, "WS"][:, dc, :],
                                     start=(dc == 0), stop=(dc == 1),
                                     skip_group_check=True)
            st[e, "b_ps"] = b_ps

        def ph_exp(e, j):
            e_all = sm_pool.tile([128, n_tiles, N], F32, tag="e_all")
            nc.scalar.activation(e_all[:], st[e, "b_ps"][:], ACT.Exp)
            st[e, "e_all"] = e_all

        def ph_ssum(e, j):
            s_sum = sm_pool.tile([128, n_tiles], F32, tag="s_sum")
            nc.vector.tensor_reduce(s_sum[:], st[e, "e_all"][:],
                                    axis=mybir.AxisListType.X, op=A.add)
            st[e, "s_sum"] = s_sum

        def ph_srecip(e, j):
            s_r = sm_pool.tile([128, n_tiles], F32, tag="s_r")
            nc.vector.reciprocal(s_r[:], st[e, "s_sum"][:])
            st[e, "s_r"] = s_r

        def ph_cmul(e, j):
            c_new = c_pool.tile([128, n_tiles, N], BF16, tag="c_all")
            nc.gpsimd.tensor_mul(c_new[:], st[e, "e_all"][:],
                                 st[e, "s_r"][:].to_broadcast(
                                     [128, n_tiles, N]))
            c_t[e] = c_new

        def ph_extract(e, j):
            nc.vector.tensor_reduce(
                out_stage[:, K * e:K * (e + 1)],
                st[e, "o_n"][:].rearrange("p (g k) -> p k g", k=K),
                axis=mybir.AxisListType.X, op=A.add)

        def routing(j, E, mid_hooks=()):
            hooks = list(mid_hooks) + [None] * 8
            for e in E:
                ph_G(e, j)
            if hooks[0]:
                hooks[0]()
            for e in E:
                ph_Gcopy(e, j)
            for e in E:
                ph_acc(e, j)
            for e in E:
                ph_om(e, j)
            for e in E:
                ph_sq(e, j)
            for e in E:
                ph_rsqrt(e, j)
            for e in E:
                ph_on(e, j)
            if j == routings - 1:
                for e in E:
                    ph_extract(e, j)
                return
            for e in E:
                ph_S(e, j)
            for e in E:
                ph_Scopy(e, j)
            for e in E:
                ph_WS(e, j)
            for e in E:
                ph_WScopy(e, j)
            # pair b (PE) with exp (Act) so the two PSUM "b" banks recycle
            order = []
            for idx, e in enumerate(E):
                order.append(("b", e))
                if idx >= 1:
                    order.append(("exp", E[idx - 1]))
            order.append(("exp", E[-1]))
            for kind, e in order:
                (ph_b if kind == "b" else ph_exp)(e, j)
            if hooks[1]:
                hooks[1]()
            for e in E:
                ph_ssum(e, j)
            for e in E:
                ph_srecip(e, j)
            for e in E:
                ph_cmul(e, j)

        # ======== emission schedule ========
        for e in range(n_ex):
            emit_dma(e)
        emit_transpose(0)
        emit_transpose(1)
        # j=0 for examples {0,1}; transposes for {2,3} fill PE stall windows
        routing(0, [0, 1], mid_hooks=[lambda: emit_transpose(2),
                                      lambda: emit_transpose(3)])
        routing(0, [2, 3])
        for j in range(1, routings):
            routing(j, list(range(n_ex)))

        # ======== store outputs ========
        nc.sync.dma_start(out_d.ap().rearrange("e n k -> n e k"),
                          out_stage[:].rearrange("p (e k) -> p e k", k=K))

    nc.compile()
    return nc


_NC_CACHE = {}


def _get_nc(**kw):
    key = tuple(sorted(kw.items()))
    if key not in _NC_CACHE:
        _NC_CACHE[key] = build_kernel(**kw)
    return _NC_CACHE[key]


def make_const_inputs():
    ones16 = np.full((128, N), 1.0 / N, dtype=ml_dtypes.bfloat16)
    bmask = np.zeros((N, M), dtype=np.float32)
    for n in range(N):
        bmask[n, n * K:(n + 1) * K] = 1.0
    id16 = np.eye(N, dtype=np.float32)
    id128 = np.eye(128, dtype=ml_dtypes.bfloat16)
    return ones16, bmask, id16, id128


def kernel(x, W, num_capsule=None, dim_capsule=None, routings=None, **_):
    x = np.asarray(x, dtype=np.float32)
    W = np.asarray(W, dtype=np.float32)
    assert x.shape == (B, IN, D), x.shape

    nc = _get_nc()
    ones16, bmask, id16, id128 = make_const_inputs()
    Wtb = np.ascontiguousarray(W[0].reshape(2, 128, M)).astype(ml_dtypes.bfloat16)
    WT = np.ascontiguousarray(W[0].T)  # [m, d]
    WTtb = np.ascontiguousarray(
        WT.reshape(2, 128, 2, 128).transpose(0, 2, 1, 3)).astype(
            ml_dtypes.bfloat16)

    n_per = B // N_CORES
    in_maps = []
    for c in range(N_CORES):
        xs = x[c * n_per:(c + 1) * n_per]              # [4, 4096, 256]
        xT = np.ascontiguousarray(
            xs.transpose(0, 2, 1)).reshape(n_per, 2, 128, IN).astype(
                ml_dtypes.bfloat16)
        in_maps.append({"xT": xT, "Wt": Wtb, "WTt": WTtb, "id128": id128,
                        "ones16": ones16, "bmask": bmask, "id16": id16})

    res = run_bass_kernel_spmd(nc, in_maps, core_ids=list(range(N_CORES)))
    out = np.concatenate([r["out"] for r in res.results], axis=0)
    return out.astype(np.float32)


# revision 10
# speedup vs baseline: 1.1611x; 1.0346x over previous
"""Capsule-routing kernel for Trainium2, 8-core batch-parallel.

Reference computation (per example, In=4096, D=256, N=16, K=16, routings=3):
    u_hat = (x @ W).reshape(In, N, K)            # [In, 256] with m = n*16+k
    b = 0
    for j in range(3):
        c = softmax(b, axis=n)                   # [In, N]
        outputs = squash(sum_i c[i,n] u_hat[i,n,:])   # [N, K]
        if j < 2: b[i,n] = sum_k outputs[n,k] u_hat[i,n,k]

Key algebraic restructure: u_hat is never materialized.
    acc = C^T (X W) = (C^T X) W        -> G = X^T C  [D,16], acc = G^T W  [16,256]
    b   = (X W) S   = X (W S)          -> WS = WT-tiles @ S [D,16], b = X WS
so the only big PE work is 64 tile transposes of xT (bf16) per example to
get x in [i,d] layout; the routing itself is ~7k PE cycles/example.

Schedule: all 4 examples' routing is emitted phase-interleaved so that
cross-engine dependency latency amortizes 4-wide and every engine queue
always has ready work behind a stalled head (engines execute in order).
"""

import sys
from contextlib import ExitStack

sys.path.insert(0, "/opt/trn_rl_repo")

import numpy as np
import ml_dtypes

import concourse.bass as bass
import concourse.mybir as mybir
import concourse.tile as tile
from concourse import bacc
from concourse.bass_utils import run_bass_kernel_spmd

F32 = mybir.dt.float32
BF16 = mybir.dt.bfloat16
U32 = mybir.dt.uint32

N_CORES = 8
B = 32
IN = 4096
D = 256
N = 16
K = 16
M = N * K  # 256
EPS = 1e-7


def build_kernel(n_ex=4, n_tiles=32, routings=3,
                 copy_rot=("v", "a", "v", "a", "v", "a", "v", "a")):
    """Build the per-core Bass module. In = n_tiles*128."""
    In = n_tiles * 128
    nc = bacc.Bacc("TRN2", target_bir_lowering=False, debug=False,
                   num_devices=N_CORES)

    # DRAM I/O
    CW = 2 * M + 4 * 128 + 128 + N  # bf16 const row width per partition
    xT_d = nc.dram_tensor("xT", [n_ex, 2, 128, In], BF16, kind="ExternalInput")
    cb_d = nc.dram_tensor("cb", [128, CW], BF16, kind="ExternalInput")
    cf_d = nc.dram_tensor("cf", [N, M + N], F32, kind="ExternalInput")
    out_d = nc.dram_tensor("out", [n_ex, N, K], F32, kind="ExternalOutput")

    with tile.TileContext(nc) as tc, ExitStack() as ctx:
        # ---- pools ----
        const_pool = ctx.enter_context(tc.tile_pool(name="consts", bufs=1))
        xT_pool = ctx.enter_context(tc.tile_pool(name="xT", bufs=n_ex))
        x_pool = ctx.enter_context(tc.tile_pool(name="x", bufs=n_ex))
        c_pool = ctx.enter_context(tc.tile_pool(name="c", bufs=n_ex))
        sm_pool = ctx.enter_context(tc.tile_pool(name="sm", bufs=n_ex))
        small_pool = ctx.enter_context(tc.tile_pool(name="small", bufs=n_ex))
        out_pool = ctx.enter_context(tc.tile_pool(name="outstage", bufs=1))
        # single PSUM pool; per-tag bufs keep the total at 13.5KB <= 8 banks
        ps = ctx.enter_context(tc.tile_pool(name="ps", bufs=1, space="PSUM"))

        # ---- constants (one DMA each for bf16 / f32 packs) ----
        cb = const_pool.tile([128, CW], BF16, tag="cb")
        cf = const_pool.tile([N, M + N], F32, tag="cf")
        Wt = cb[:, 0:2 * M].rearrange("p (c m) -> p c m", m=M)
        WTt = cb[:, 2 * M:2 * M + 4 * 128].rearrange(
            "p (a b f) -> p a b f", b=2, f=128)
        id128 = cb[:, 2 * M + 4 * 128:2 * M + 4 * 128 + 128]
        ones16 = cb[:, 2 * M + 4 * 128 + 128:]
        bmask = cf[:, 0:M]
        id16 = cf[:, M:]

        out_stage = out_pool.tile([N, n_ex * K], F32, tag="outst")

        A = mybir.AluOpType
        ACT = mybir.ActivationFunctionType

        xT_t = [None] * n_ex
        x_t = [None] * n_ex
        c_t = [None] * n_ex
        st = {}  # per-example per-stage small tiles

        def emit_dma(e):
            xT = xT_pool.tile([128, 2, In], BF16, tag="xT")
            nc.sync.dma_start(xT[:, 0, :], xT_d[e, 0])
            nc.sync.dma_start(xT[:, 1, :], xT_d[e, 1])
            xT_t[e] = xT

        def emit_transpose(e):
            """xT [d,i] tiles -> x [i,d] tiles via PE, staged through PSUM."""
            xT = xT_t[e]
            x_sb = x_pool.tile([128, n_tiles, 2, 128], BF16, tag="x")
            pairs = [(t, dc) for t in range(n_tiles) for dc in range(2)]
            for bi in range(len(pairs) // 8):
                stage = ps.tile([128, 8, 128], BF16, tag="stage", bufs=2)
                for s in range(8):
                    t, dc = pairs[8 * bi + s]
                    nc.tensor.matmul(stage[:, s, :],
                                     xT[:, dc, 128 * t:128 * (t + 1)],
                                     id128, is_transpose=True,
                                     start=True, stop=True,
                                     skip_group_check=True)
                dst = x_sb[:, 4 * bi:4 * (bi + 1), :, :].rearrange(
                    "p a b f -> p (a b f)")
                if copy_rot[bi % len(copy_rot)] == "v":
                    nc.vector.tensor_copy(dst, stage[:])
                else:
                    nc.scalar.copy(dst, stage[:])
            x_t[e] = x_sb

        # ---------- routing phases (each emitted for all examples) ----------
        def ph_G(e, j):
            # one PSUM bank per routing step: g [0:32], S [32:64], WS [64:96],
            # acc (partitions 0:16) [96:352]
            misc = ps.tile([128, 512], F32, tag="misc", bufs=4)
            st[e, "misc"] = misc
            for dc in range(2):
                for t in range(n_tiles):
                    c_ap = ones16 if j == 0 else c_t[e][:, t, :]
                    nc.tensor.matmul(misc[:, 16 * dc:16 * (dc + 1)],
                                     x_t[e][:, t, dc, :], c_ap,
                                     start=(t == 0), stop=(t == n_tiles - 1),
                                     skip_group_check=True)

        def ph_Gcopy(e, j):
            G_sb = small_pool.tile([128, 2, N], BF16, tag="G")
            nc.vector.tensor_copy(G_sb[:].rearrange("p a b -> p (a b)"),
                                  st[e, "misc"][:, 0:32])
            st[e, "G"] = G_sb

        def ph_acc(e, j):
            acc_ps = st[e, "misc"][0:N, 96:96 + M]
            for dc in range(2):
                nc.tensor.matmul(acc_ps, st[e, "G"][:, dc, :], Wt[:, dc, :],
                                 start=(dc == 0), stop=(dc == 1),
                                 skip_group_check=True)
            st[e, "acc"] = acc_ps

        def ph_om(e, j):
            om = small_pool.tile([N, M], F32, tag="om")
            nc.vector.tensor_mul(om[:], st[e, "acc"], bmask)
            st[e, "om"] = om

        def ph_sq(e, j):
            nrm2 = small_pool.tile([N, 1], F32, tag="nrm2")
            sq = small_pool.tile([N, M], F32, tag="sq")
            nc.scalar.activation(sq[:], st[e, "om"][:], ACT.Square,
                                 accum_out=nrm2[:])
            st[e, "nrm2"] = nrm2

        def rsqrt_steps(e, j):
            # rinv = 1/sqrt(nrm2 + eps) via bit-trick + Newton steps (DVE only;
            # Act Sqrt would thrash the activation table against Exp).
            # Returned as op thunks so the emitter can interleave examples.
            xe = small_pool.tile([N, 1], F32, tag="xe")
            sbits = small_pool.tile([N, 1], U32, tag="sbits")
            ybits = small_pool.tile([N, 1], U32, tag="ybits")
            t1 = small_pool.tile([N, 1], F32, tag="t1")
            t2 = small_pool.tile([N, 1], F32, tag="t2")
            rinv = small_pool.tile([N, 1], F32, tag="rinv")
            st[e, "rinv"] = rinv
            ops = [
                lambda: nc.vector.tensor_scalar_add(xe[:], st[e, "nrm2"][:], EPS),
                lambda: nc.vector.tensor_scalar(
                    sbits[:], xe[:].bitcast(U32), 1, None,
                    op0=A.logical_shift_right),
                lambda: nc.vector.tensor_scalar(
                    ybits[:], sbits[:], -1.0, float(0x5F3759DF),
                    op0=A.mult, op1=A.add),
            ]
            ys = [ybits[:].bitcast(F32)]
            n_newton = 2 if j == routings - 1 else 1
            for it in range(n_newton):
                dst = rinv if it == n_newton - 1 else small_pool.tile(
                    [N, 1], F32, tag="ynext")
                def mk(it=it, dst=dst):
                    y = ys[-1]
                    ops.append(lambda: nc.vector.tensor_mul(t1[:], xe[:], y))
                    ops.append(lambda: nc.vector.tensor_mul(t2[:], t1[:], y))
                    ops.append(lambda: nc.vector.tensor_scalar(
                        t2[:], t2[:], -0.5, 1.5, op0=A.mult, op1=A.add))
                    ops.append(lambda: nc.vector.tensor_mul(dst[:], t2[:], y))
                    ys.append(dst[:])
                mk()
            return ops

        def ph_on(e, j):
            o_n = small_pool.tile([N, M], F32, tag="o_n")
            nc.gpsimd.tensor_scalar_mul(o_n[:], st[e, "om"][:],
                                        st[e, "rinv"][:])
            st[e, "o_n"] = o_n

        def ph_S(e, j):
            misc = st[e, "misc"]
            for mc in range(2):
                nc.tensor.transpose(misc[:, 32 + 16 * mc:32 + 16 * (mc + 1)],
                                    st[e, "o_n"][:, 128 * mc:128 * (mc + 1)],
                                    id16)

        def ph_Scopy(e, j):
            S_sb = small_pool.tile([128, 2, N], BF16, tag="S")
            nc.vector.tensor_copy(S_sb[:].rearrange("p a b -> p (a b)"),
                                  st[e, "misc"][:, 32:64])
            st[e, "S"] = S_sb

        def ph_WS(e, j):
            misc = st[e, "misc"]
            for dc in range(2):
                for mc in range(2):
                    nc.tensor.matmul(misc[:, 64 + 16 * dc:64 + 16 * (dc + 1)],
                                     WTt[:, mc, dc, :],
                                     st[e, "S"][:, mc, :],
                                     start=(mc == 0), stop=(mc == 1),
                                     skip_group_check=True)

        def ph_WScopy(e, j):
            WS_sb = small_pool.tile([128, 2, N], BF16, tag="WS")
            nc.vector.tensor_copy(WS_sb[:].rearrange("p a b -> p (a b)"),
                                  st[e, "misc"][:, 64:96])
            st[e, "WS"] = WS_sb

        def ph_b(e, j):
            b_ps = ps.tile([128, n_tiles, N], F32, tag="b", bufs=2)
            for t in range(n_tiles):
                for dc in range(2):
                    nc.tensor.matmul(b_ps[:, t, :],
                                     xT_t[e][:, dc, 128 * t:128 * (t + 1)],
                                     st[e, "WS"][:, dc, :],
                                     start=(dc == 0), stop=(dc == 1),
                                     skip_group_check=True)
            st[e, "b_ps"] = b_ps

        def ph_exp(e, j):
            e_all = sm_pool.tile([128, n_tiles, N], F32, tag="e_all")
            nc.scalar.activation(e_all[:], st[e, "b_ps"][:], ACT.Exp)
            st[e, "e_all"] = e_all

        def ph_ssum(e, j):
            s_sum = sm_pool.tile([128, n_tiles], F32, tag="s_sum")
            nc.vector.tensor_reduce(s_sum[:], st[e, "e_all"][:],
                                    axis=mybir.AxisListType.X, op=A.add)
            st[e, "s_sum"] = s_sum

        def ph_srecip(e, j):
            s_r = sm_pool.tile([128, n_tiles], F32, tag="s_r")
            nc.vector.reciprocal(s_r[:], st[e, "s_sum"][:])
            st[e, "s_r"] = s_r

        def ph_cmul(e, j):
            c_new = c_pool.tile([128, n_tiles, N], BF16, tag="c_all")
            nc.gpsimd.tensor_mul(c_new[:], st[e, "e_all"][:],
                                 st[e, "s_r"][:].to_broadcast(
                                     [128, n_tiles, N]))
            c_t[e] = c_new

        def ph_extract(e, j):
            nc.vector.tensor_reduce(
                out_stage[:, K * e:K * (e + 1)],
                st[e, "o_n"][:].rearrange("p (g k) -> p k g", k=K),
                axis=mybir.AxisListType.X, op=A.add)

        def routing(j, E, mid_hooks=()):
            hooks = list(mid_hooks) + [None] * 8
            for e in E:
                ph_G(e, j)
            if hooks[0]:
                hooks[0]()
            for e in E:
                ph_Gcopy(e, j)
            for e in E:
                ph_acc(e, j)
            for e in E:
                ph_om(e, j)
            for e in E:
                ph_sq(e, j)
            all_ops = [rsqrt_steps(e, j) for e in E]
            for oi in range(max(len(o) for o in all_ops)):
                for ops in all_ops:
                    if oi < len(ops):
                        ops[oi]()
            for e in E:
                ph_on(e, j)
            if j == routings - 1:
                for e in E:
                    ph_extract(e, j)
                return
            for e in E:
                ph_S(e, j)
            for e in E:
                ph_Scopy(e, j)
            for e in E:
                ph_WS(e, j)
            for e in E:
                ph_WScopy(e, j)
            # pair b (PE) with exp (Act) so the two PSUM "b" banks recycle
            order = []
            for idx, e in enumerate(E):
                order.append(("b", e))
                if idx >= 1:
                    order.append(("exp", E[idx - 1]))
            order.append(("exp", E[-1]))
            for kind, e in order:
                (ph_b if kind == "b" else ph_exp)(e, j)
            if hooks[1]:
                hooks[1]()
            for e in E:
                ph_ssum(e, j)
            for e in E:
                ph_srecip(e, j)
            for e in E:
                ph_cmul(e, j)

        # ======== emission schedule ========
        emit_dma(0)
        nc.sync.dma_start(cb[:], cb_d[:])
        nc.sync.dma_start(cf[:], cf_d[:])
        for e in range(1, n_ex):
            emit_dma(e)
        emit_transpose(0)
        emit_transpose(1)
        # j=0 for examples {0,1}; transposes for {2,3} fill PE stall windows
        routing(0, [0, 1], mid_hooks=[lambda: emit_transpose(2),
                                      lambda: emit_transpose(3)])
        routing(0, [2, 3])
        for j in range(1, routings):
            routing(j, list(range(n_ex)))

        # ======== store outputs ========
        nc.sync.dma_start(out_d.ap().rearrange("e n k -> n e k"),
                          out_stage[:].rearrange("p (e k) -> p e k", k=K))

    nc.compile()
    return nc


_NC_CACHE = {}


def _get_nc(**kw):
    key = tuple(sorted(kw.items()))
    if key not in _NC_CACHE:
        _NC_CACHE[key] = build_kernel(**kw)
    return _NC_CACHE[key]


def make_const_inputs(W):
    """Pack constants: cb [128, CW] bf16 and cf [N, M+N] f32."""
    Wtb = W[0].reshape(2, 128, M)                     # [c, p, m]
    WT = np.ascontiguousarray(W[0].T)                 # [m, d]
    WTtb = WT.reshape(2, 128, 2, 128).transpose(0, 2, 1, 3)  # [a, b, p, f]
    cb = np.concatenate([
        Wtb.transpose(1, 0, 2).reshape(128, 2 * M),
        WTtb.transpose(2, 0, 1, 3).reshape(128, 4 * 128),
        np.eye(128, dtype=np.float32),
        np.full((128, N), 1.0 / N, dtype=np.float32),
    ], axis=1).astype(ml_dtypes.bfloat16)
    bmask = np.zeros((N, M), dtype=np.float32)
    for n in range(N):
        bmask[n, n * K:(n + 1) * K] = 1.0
    cf = np.concatenate([bmask, np.eye(N, dtype=np.float32)], axis=1)
    return cb, cf


def kernel(x, W, num_capsule=None, dim_capsule=None, routings=None, **_):
    x = np.asarray(x, dtype=np.float32)
    W = np.asarray(W, dtype=np.float32)
    assert x.shape == (B, IN, D), x.shape

    nc = _get_nc()
    cb, cf = make_const_inputs(W)

    n_per = B // N_CORES
    in_maps = []
    for c in range(N_CORES):
        xs = x[c * n_per:(c + 1) * n_per]              # [4, 4096, 256]
        xT = np.ascontiguousarray(
            xs.transpose(0, 2, 1)).reshape(n_per, 2, 128, IN).astype(
                ml_dtypes.bfloat16)
        in_maps.append({"xT": xT, "cb": cb, "cf": cf})

    res = run_bass_kernel_spmd(nc, in_maps, core_ids=list(range(N_CORES)))
    out = np.concatenate([r["out"] for r in res.results], axis=0)
    return out.astype(np.float32)
